# revision 2
# baseline (speedup 1.0000x reference)
"""Trainium2 Bass kernel for the 4-DOF arm dynamics step (nn_Arm_3D_Dyn).

Strategy: pure data-parallel over the 1M rows across 8 NeuronCores.
Per core: rows are laid out [128 partitions x 977 rows-per-partition]
(padded), processed in free-dim chunks. All trig is done on ScalarE
(sin LUT, cos via the free +pi/2 affine); the inertia-matrix / Coriolis
assembly uses a hand-derived bracket decomposition (~190 DVE ops/row
after scalar folding); the 4x4 SPD solve is a permuted LDLT
(elimination order [3,2,0,1]) that exploits d23=0 and constant d33.
Scalar parameters L1,L2,M1,M2 are baked into instruction immediates at
build time; all scalar coefficients ride for free in scalar_tensor_tensor
/ tensor_scalar slots via lazy scale tracking.
"""
import numpy as np

DT_STEP = 0.01
LAM = 2.0
N_TOTAL = 1_000_000
NCORES = 8
ROWS_PER_CORE = N_TOTAL // NCORES          # 125_000
RPP = (ROWS_PER_CORE + 127) // 128          # 977 rows per partition
PADDED = 128 * RPP                          # 125_056
CHUNKS = [326, 326, 325]
assert sum(CHUNKS) == RPP

ENG_OPS = ('sin', 'mul', 'add', 'sub', 'stt', 'ts', 'recip')


class _Ref:
    __slots__ = ('name', 's')
    def __init__(self, name, s=1.0):
        self.name = name
        self.s = float(s)


class _Builder:
    def __init__(self):
        self.ops = []
        self.n = 0
    def _new(self):
        self.n += 1
        return f"v{self.n}"
    def inp(self, name):
        return _Ref(name, 1.0)
    def sin(self, a, bias=0.0):
        assert abs(a.s - 1.0) < 1e-12
        o = self._new(); self.ops.append(('sin', o, a.name, float(bias)))
        return _Ref(o, 1.0)
    def mul(self, a, b):
        o = self._new(); self.ops.append(('mul', o, a.name, b.name))
        return _Ref(o, a.s * b.s)
    def sq(self, a):
        return self.mul(a, a)
    def smul(self, a, s):
        return _Ref(a.name, a.s * s)
    def add(self, a, b):
        if a.s == b.s:
            o = self._new(); self.ops.append(('add', o, a.name, b.name))
            return _Ref(o, a.s)
        o = self._new()
        self.ops.append(('stt', o, a.name, a.s / b.s, 'mult', b.name, 'add'))
        return _Ref(o, b.s)
    def sub(self, a, b):
        if a.s == b.s:
            o = self._new(); self.ops.append(('sub', o, a.name, b.name))
            return _Ref(o, a.s)
        o = self._new()
        self.ops.append(('stt', o, a.name, a.s / b.s, 'mult', b.name, 'subtract'))
        return _Ref(o, b.s)
    def sadd(self, a, const):
        o = self._new()
        self.ops.append(('ts', o, a.name, a.s, 'mult', float(const), 'add'))
        return _Ref(o, 1.0)
    def affine(self, a, m, c):
        o = self._new()
        self.ops.append(('ts', o, a.name, a.s * m, 'mult', float(c), 'add'))
        return _Ref(o, 1.0)
    def recip(self, a):
        o = self._new(); self.ops.append(('recip', o, a.name))
        return _Ref(o, 1.0 / a.s)


def build_ir(l1, l2, m1, m2):
    bl = _Builder()
    t = bl.inp
    PI2 = float(np.pi / 2)
    s2 = bl.sin(t('th2')); c2 = bl.sin(t('th2'), PI2)
    s3 = bl.sin(t('th3')); c3 = bl.sin(t('th3'), PI2)
    s4 = bl.sin(t('th4')); c4 = bl.sin(t('th4'), PI2)
    U = bl.mul(c3, c4); V = bl.mul(s3, c4); W = bl.mul(s3, s4); Z = bl.mul(c3, s4)
    c22 = bl.sq(c2); C42 = bl.sq(c4)
    c2s4 = bl.mul(c2, s4); s2U = bl.mul(s2, U); P = bl.add(s2U, c2s4)
    c2c4 = bl.mul(c2, c4); s2Z = bl.mul(s2, Z); Q = bl.sub(c2c4, s2Z)
    S2S4 = bl.mul(s2, s4); C2U = bl.mul(c2, U)
    A2 = bl.mul(s2, c2)
    V2 = bl.sq(V)
    S3V = bl.mul(s3, V)
    US4 = bl.mul(U, s4)
    WS4 = bl.mul(W, s4)
    C4S4 = bl.mul(c4, s4)
    C4V = bl.mul(c4, V)
    C2V = bl.mul(c2, V)
    C2C3 = bl.mul(c2, c3)
    S2V2 = bl.mul(s2, V2)
    S2S3 = bl.mul(s2, s3); S2S3q = bl.sq(S2S3)
    Qq = bl.sq(Q)
    Rm = bl.sub(C2U, S2S4)
    C2Rm = bl.mul(c2, Rm)
    qq = bl.add(Qq, S2S3q)
    c2t2 = bl.affine(c22, 2.0, -1.0)
    c2t4 = bl.affine(C42, 2.0, -1.0)
    S2P = bl.mul(s2, P)
    K   = bl.sub(bl.smul(S2P, 2*l2), bl.smul(c22, 3*l1))
    E0  = bl.sub(bl.smul(c2, 3*l1), bl.smul(S2S4, 2*l2))
    E2  = bl.add(E0, bl.smul(C2U, 2*l2))
    B   = bl.add(bl.smul(s2, 3*l1), bl.smul(P, 2*l2))
    F1  = bl.affine(U, 2*l2, 3*l1)
    F2  = bl.sub(bl.smul(c3, 3*l1), bl.smul(S3V, 2*l2))
    E02 = bl.add(bl.smul(C2C3, 3*l1), bl.smul(Q, 2*l2))
    aW, bW, cW = 2*l2*l2*m2, 6*l1*l2*m2, 2*l1*l1*(m1+3*m2)
    U2 = bl.sq(U)
    w1 = bl.add(bl.smul(U2, aW), bl.smul(U, bW))
    w2 = bl.add(bl.smul(C42, aW), w1)
    W2 = bl.sadd(w2, cW - aW)
    p1 = bl.mul(F1, s4)
    p1c = bl.mul(p1, c2t2)
    p2 = bl.mul(A2, W2)
    GG = bl.add(bl.smul(p1c, l2*m2), p2)
    x1 = bl.mul(c3, F1)
    i1 = bl.add(x1, bl.smul(c4, 2*l2))
    x2 = bl.mul(c3, c2t4)
    i2 = bl.add(bl.smul(x2, 2*l2), bl.smul(c4, 3*l1))
    S4C22 = bl.mul(s4, c22)
    b1 = bl.mul(S4C22, i1)
    b2 = bl.mul(A2, i2)
    C3US4 = bl.mul(c3, US4)
    z1 = bl.add(b1, b2)
    B14 = bl.sub(z1, bl.smul(C3US4, 2*l2))
    VK = bl.mul(V, K); VE2 = bl.mul(V, E2); VE0 = bl.mul(V, E0); VQ = bl.mul(V, Q)
    VF1 = bl.mul(V, F1); S4F2 = bl.mul(s4, F2); BU = bl.mul(B, U); BW = bl.mul(B, W)
    VB = bl.mul(V, B); S4E02 = bl.mul(s4, E02); C4P = bl.mul(c4, P)
    e1 = bl.add(bl.smul(c22, l1*l1*m1/3 + l1*l1*m2), bl.smul(C2Rm, l1*l2*m2))
    d00 = bl.add(e1, bl.smul(qq, l2*l2*m2/3))
    d01 = bl.smul(VB, l2*m2/6)
    d02 = bl.smul(bl.mul(c4, E02), l2*m2/6)
    i3 = bl.add(bl.smul(s2, 2*l2), bl.smul(c2s4, -3*l1))
    d03 = bl.smul(bl.mul(s3, i3), l2*m2/6)
    e3 = bl.add(bl.smul(U, l1*l2*m2), bl.smul(V2, -l2*l2*m2/3))
    d11 = bl.sadd(e3, l1*l1*m1/3 + l1*l1*m2 + l2*l2*m2/3)
    d12 = bl.smul(bl.mul(V, s4), l2*l2*m2/3)
    d13 = bl.add(bl.smul(c4, l1*l2*m2/2), bl.smul(c3, l2*l2*m2/3))
    d22 = bl.smul(C42, l2*l2*m2/3)
    g3 = l2*l2*m2/3
    q = {}
    for i_ in range(1, 5):
        for j_ in range(i_, 5):
            q[(i_, j_)] = bl.mul(t(f'dt{i_}'), t(f'dt{j_}'))
    lm, l2m = l2*m2, l2*l2*m2
    hterms = {
     0: [((1,2), GG, -1.0/3), ((1,3), VK, lm/3), ((1,4), B14, -lm/3),
         ((2,2), VE2, lm/6), ((2,3), S2V2, -2*l2m/3), ((2,4), VQ, 2*l2m/3),
         ((3,3), VE0, -lm/6), ((3,4), S4E02, -lm/3), ((4,4), C2V, -l1*lm/2)],
     1: [((1,1), GG, 1.0/6), ((1,3), BU, lm/3), ((1,4), BW, -lm/3),
         ((2,3), VF1, -lm/3), ((2,4), S4F2, -lm/3), ((3,3), US4, l2m/3),
         ((3,4), WS4, -2*l2m/3), ((4,4), s4, -l1*lm/2)],
     2: [((1,1), VK, -lm/6), ((1,2), BU, -lm/3), ((1,4), C4P, -2*l2m/3),
         ((2,2), VF1, lm/6), ((2,4), C4V, 2*l2m/3), ((3,4), C4S4, -2*l2m/3)],
     3: [((1,1), B14, lm/6), ((1,2), BW, lm/3), ((1,3), C4P, 2*l2m/3),
         ((2,2), S4F2, lm/6), ((2,3), C4V, -2*l2m/3), ((3,3), C4S4, l2m/3)],
    }
    rhs = []
    for k in range(4):
        acc = t(f'ta{k+1}')
        for (qp, br, cf) in hterms[k]:
            term = bl.mul(q[qp], br)
            acc = bl.add(bl.smul(term, -cf), acc)
        rhs.append(acc)
    a_, b_, c_, d_, e_, f_, g_, h_ = d00, d01, d02, d03, d11, d12, d13, d22
    inv3 = 1.0 / g3
    a00 = bl.sub(a_, bl.smul(bl.sq(d_), inv3))
    a01 = bl.sub(b_, bl.smul(bl.mul(d_, g_), inv3))
    a11 = bl.sub(e_, bl.smul(bl.sq(g_), inv3))
    r2 = bl.recip(h_)
    l02 = bl.mul(c_, r2); l12 = bl.mul(f_, r2)
    b00 = bl.sub(a00, bl.mul(l02, c_))
    b01 = bl.sub(a01, bl.mul(l12, c_))
    b11 = bl.sub(a11, bl.mul(l12, f_))
    r0p = bl.recip(b00)
    l01 = bl.mul(b01, r0p)
    c11 = bl.sub(b11, bl.mul(l01, b01))
    r1p = bl.recip(c11)
    y3, y2 = rhs[3], rhs[2]
    y0 = bl.sub(bl.sub(rhs[0], bl.smul(bl.mul(d_, y3), inv3)), bl.mul(l02, y2))
    y1 = bl.sub(bl.sub(bl.sub(rhs[1], bl.smul(bl.mul(g_, y3), inv3)),
                       bl.mul(l12, y2)), bl.mul(l01, y0))
    z3 = bl.smul(y3, inv3); z2 = bl.mul(y2, r2)
    z0 = bl.mul(y0, r0p);   z1 = bl.mul(y1, r1p)
    x1s = z1
    x0s = bl.sub(z0, bl.mul(l01, x1s))
    x2s = bl.sub(bl.sub(z2, bl.mul(l02, x0s)), bl.mul(l12, x1s))
    x3s = bl.sub(bl.sub(z3, bl.smul(bl.mul(d_, x0s), inv3)),
                 bl.smul(bl.mul(g_, x1s), inv3))
    bl.ops.append(('out_theta',))
    bl.ops.append(('out_tau',))
    for k, xk in enumerate([x0s, x1s, x2s, x3s]):
        bl.ops.append(('out_vel', k, xk.name, DT_STEP * xk.s))
    return bl.ops


def _alloc_registers(ops):
    """Linear-scan register allocation over the IR. Returns (reg_of, nregs)."""
    last_use = {}
    defs = set()
    for i, op in enumerate(ops):
        if op[0] in ENG_OPS:
            defs.add(op[1])
            for a in op[2:]:
                if isinstance(a, str) and a in defs:
                    last_use[a] = i
        elif op[0] == 'out_vel':
            last_use[op[2]] = i
    free = []
    reg_of = {}
    nregs = 0
    live = set()
    for i, op in enumerate(ops):
        if op[0] not in ENG_OPS:
            continue
        for nm in [n for n in live if last_use.get(n, -1) < i]:
            live.discard(nm)
            free.append(reg_of[nm])
        o = op[1]
        if o in last_use:
            if free:
                r = free.pop()
            else:
                r = nregs
                nregs += 1
            reg_of[o] = r
            live.add(o)
    return reg_of, nregs


def _register_const(nc, mybir, value, dtype=None):
    dtype = dtype or mybir.dt.float32
    t = nc.alloc_sbuf_tensor(f"const-{dtype.name}-{value}", [128, 1], dtype)
    nc.gpsimd.memset(t.ap(), value)
    nc.const_aps.aps[(dtype, value)] = t.ap()


def build_kernel(l1, l2, m1, m2):
    import sys
    if '/opt/trn_rl_repo' not in sys.path:
        sys.path.insert(0, '/opt/trn_rl_repo')
    from concourse import bacc, mybir, tile

    ops = build_ir(l1, l2, m1, m2)
    reg_of, nregs = _alloc_registers(ops)

    nc = bacc.Bacc(None)
    F32 = mybir.dt.float32
    A = mybir.AluOpType
    AF = mybir.ActivationFunctionType

    _register_const(nc, mybir, float(np.pi / 2))
    nc.all_engine_barrier()

    theta_d = nc.declare_dram_parameter("theta", [PADDED, 4], F32, isOutput=False)
    vel_d = nc.declare_dram_parameter("vel", [PADDED, 4], F32, isOutput=False)
    tau_d = nc.declare_dram_parameter("tau", [PADDED, 4], F32, isOutput=False)
    out_d = nc.declare_dram_parameter("out", [PADDED, 12], F32, isOutput=True)

    theta_r = theta_d[:].rearrange("(p r) c -> p r c", p=128)
    vel_r = vel_d[:].rearrange("(p r) c -> p r c", p=128)
    tau_r = tau_d[:].rearrange("(p r) c -> p r c", p=128)
    out_r = out_d[:].rearrange("(p r) c -> p r c", p=128)

    FMAX = max(CHUNKS)

    with tile.TileContext(nc) as tc:
        with tc.tile_pool(name="io", bufs=2) as iop, \
             tc.tile_pool(name="work", bufs=2) as wp:
            off = 0
            for F in CHUNKS:
                th_t = iop.tile([128, FMAX * 4], F32, tag="th")
                ve_t = iop.tile([128, FMAX * 4], F32, tag="ve")
                ta_t = iop.tile([128, FMAX * 4], F32, tag="ta")
                ou_t = iop.tile([128, FMAX * 12], F32, tag="ou")
                th_v = th_t[:].rearrange("p (r c) -> p r c", c=4)[:, :F, :]
                ve_v = ve_t[:].rearrange("p (r c) -> p r c", c=4)[:, :F, :]
                ta_v = ta_t[:].rearrange("p (r c) -> p r c", c=4)[:, :F, :]
                ou_v = ou_t[:].rearrange("p (r c) -> p r c", c=12)[:, :F, :]
                nc.sync.dma_start(out=th_v, in_=theta_r[:, off:off + F, :])
                nc.sync.dma_start(out=ve_v, in_=vel_r[:, off:off + F, :])
                nc.sync.dma_start(out=ta_v, in_=tau_r[:, off:off + F, :])

                regs = {}
                def rtile(name):
                    r = reg_of[name]
                    if r not in regs:
                        regs[r] = wp.tile([128, FMAX], F32, tag=f"r{r}",
                                          name=f"r{r}")
                    return regs[r][:, :F]

                def get(name):
                    if name.startswith('th'):
                        return th_v[:, :, int(name[2]) - 1]
                    if name.startswith('dt'):
                        return ve_v[:, :, int(name[2]) - 1]
                    if name.startswith('ta'):
                        return ta_v[:, :, int(name[2]) - 1]
                    return rtile(name)

                for op in ops:
                    tag = op[0]
                    if tag == 'sin':
                        _, o, a, bias = op
                        nc.scalar.activation(rtile(o), get(a), AF.Sin,
                                             bias=float(bias))
                    elif tag == 'mul':
                        _, o, a, b = op
                        nc.vector.tensor_tensor(out=rtile(o), in0=get(a),
                                                in1=get(b), op=A.mult)
                    elif tag == 'add':
                        _, o, a, b = op
                        nc.vector.tensor_tensor(out=rtile(o), in0=get(a),
                                                in1=get(b), op=A.add)
                    elif tag == 'sub':
                        _, o, a, b = op
                        nc.vector.tensor_tensor(out=rtile(o), in0=get(a),
                                                in1=get(b), op=A.subtract)
                    elif tag == 'stt':
                        _, o, a, s, op0, b, op1 = op
                        nc.vector.scalar_tensor_tensor(
                            out=rtile(o), in0=get(a), scalar=float(s),
                            in1=get(b), op0=getattr(A, op0), op1=getattr(A, op1))
                    elif tag == 'ts':
                        _, o, a, s1, op0, s2, op1 = op
                        if op1 is None:
                            nc.vector.tensor_scalar(
                                out=rtile(o), in0=get(a), scalar1=float(s1),
                                scalar2=None, op0=getattr(A, op0))
                        else:
                            nc.vector.tensor_scalar(
                                out=rtile(o), in0=get(a), scalar1=float(s1),
                                scalar2=float(s2), op0=getattr(A, op0),
                                op1=getattr(A, op1))
                    elif tag == 'recip':
                        _, o, a = op
                        nc.vector.reciprocal_approx_fast(out=rtile(o), in_=get(a))
                    elif tag == 'out_theta':
                        nc.vector.scalar_tensor_tensor(
                            out=ou_v[:, :, 0:4], in0=ve_v, scalar=DT_STEP,
                            in1=th_v, op0=A.mult, op1=A.add)
                    elif tag == 'out_tau':
                        nc.vector.tensor_scalar(
                            out=ou_v[:, :, 8:12], in0=ta_v,
                            scalar1=float(1.0 - LAM * DT_STEP), scalar2=None,
                            op0=A.mult)
                    elif tag == 'out_vel':
                        _, k, node, s = op
                        nc.vector.scalar_tensor_tensor(
                            out=ou_v[:, :, 4 + k], in0=get(node),
                            scalar=float(s), in1=ve_v[:, :, k],
                            op0=A.mult, op1=A.add)
                    else:
                        raise ValueError(tag)

                nc.sync.dma_start(out=out_r[:, off:off + F, :], in_=ou_v)
                off += F

    nc.finalize()
    return nc


_cache = {}


def _get_nc(l1, l2, m1, m2):
    key = (round(l1, 9), round(l2, 9), round(m1, 9), round(m2, 9))
    if key not in _cache:
        _cache[key] = build_kernel(l1, l2, m1, m2)
    return _cache[key]


def _shard_inputs(theta, vel, tau):
    in_maps = []
    for c in range(NCORES):
        m = {}
        for name, arr in (("theta", theta), ("vel", vel), ("tau", tau)):
            a = np.asarray(arr, dtype=np.float32)[c * ROWS_PER_CORE:(c + 1) * ROWS_PER_CORE]
            p = np.zeros((PADDED, 4), np.float32)
            p[:ROWS_PER_CORE] = a
            m[name] = p
        in_maps.append(m)
    return in_maps


def _run(nc, in_maps, trace=False, **kw):
    import sys
    if '/opt/trn_rl_repo' not in sys.path:
        sys.path.insert(0, '/opt/trn_rl_repo')
    from concourse.bass_utils import run_bass_kernel_spmd
    return run_bass_kernel_spmd(nc, in_maps, core_ids=list(range(NCORES)),
                                trace=trace, **kw)


def kernel(theta, vel, tau, L1, L2, M1, M2):
    l1 = float(np.asarray(L1).ravel()[0])
    l2 = float(np.asarray(L2).ravel()[0])
    m1 = float(np.asarray(M1).ravel()[0])
    m2 = float(np.asarray(M2).ravel()[0])
    nc = _get_nc(l1, l2, m1, m2)
    in_maps = _shard_inputs(theta, vel, tau)
    res = _run(nc, in_maps)
    out = np.concatenate(
        [res.results[c]["out"][:ROWS_PER_CORE] for c in range(NCORES)], axis=0)
    return out.astype(np.float32)


# revision 8
# speedup vs baseline: 1.2576x; 1.2576x over previous
"""Trainium2 Bass kernel for the 4-DOF arm dynamics step (nn_Arm_3D_Dyn).

Strategy: pure data-parallel over the 1M rows across 8 NeuronCores.
Per core: rows are laid out [128 partitions x 977 rows-per-partition]
(padded), processed in free-dim chunks. All trig is done on ScalarE
(sin LUT, cos via the free +pi/2 affine); the inertia-matrix / Coriolis
assembly uses a hand-derived bracket decomposition (~190 DVE ops/row
after scalar folding); the 4x4 SPD solve is a permuted LDLT
(elimination order [3,2,0,1]) that exploits d23=0 and constant d33.
Scalar parameters L1,L2,M1,M2 are baked into instruction immediates at
build time; all scalar coefficients ride for free in scalar_tensor_tensor
/ tensor_scalar slots via lazy scale tracking.
"""
import numpy as np

DT_STEP = 0.01
LAM = 2.0
N_TOTAL = 1_000_000
NCORES = 8
ROWS_PER_CORE = N_TOTAL // NCORES          # 125_000
RPP = (ROWS_PER_CORE + 127) // 128          # 977 rows per partition
PADDED = 128 * RPP                          # 125_056
CHUNKS = [326, 326, 325]
assert sum(CHUNKS) == RPP

ENG_OPS = ('sin', 'mul', 'add', 'sub', 'stt', 'ts', 'recip', 'sq')


class _Ref:
    __slots__ = ('name', 's')
    def __init__(self, name, s=1.0):
        self.name = name
        self.s = float(s)


class _Builder:
    def __init__(self):
        self.ops = []
        self.n = 0
    def _new(self):
        self.n += 1
        return f"v{self.n}"
    def inp(self, name):
        return _Ref(name, 1.0)
    def sin(self, a, bias=0.0):
        assert abs(a.s - 1.0) < 1e-12
        o = self._new(); self.ops.append(('sin', o, a.name, float(bias), 'a'))
        return _Ref(o, 1.0)
    def mul(self, a, b, eng='v'):
        o = self._new(); self.ops.append(('mul', o, a.name, b.name, eng))
        return _Ref(o, a.s * b.s)
    def sq(self, a, eng='v'):
        if eng == 'a':
            o = self._new(); self.ops.append(('sq', o, a.name, eng))
            return _Ref(o, a.s * a.s)
        return self.mul(a, a, eng)
    def smul(self, a, s):
        return _Ref(a.name, a.s * s)
    def add(self, a, b, eng='v'):
        if a.s == b.s:
            o = self._new(); self.ops.append(('add', o, a.name, b.name, eng))
            return _Ref(o, a.s)
        o = self._new()
        self.ops.append(('stt', o, a.name, a.s / b.s, 'mult', b.name, 'add', 'v'))
        return _Ref(o, b.s)
    def sub(self, a, b, eng='v'):
        if a.s == b.s:
            o = self._new(); self.ops.append(('sub', o, a.name, b.name, eng))
            return _Ref(o, a.s)
        o = self._new()
        self.ops.append(('stt', o, a.name, a.s / b.s, 'mult', b.name, 'subtract', 'v'))
        return _Ref(o, b.s)
    def sadd(self, a, const, eng='a'):
        o = self._new()
        self.ops.append(('ts', o, a.name, a.s, 'mult', float(const), 'add', eng))
        return _Ref(o, 1.0)
    def affine(self, a, m, c, eng='a'):
        o = self._new()
        self.ops.append(('ts', o, a.name, a.s * m, 'mult', float(c), 'add', eng))
        return _Ref(o, 1.0)
    def recip(self, a):
        o = self._new(); self.ops.append(('recip', o, a.name, 'v'))
        return _Ref(o, 1.0 / a.s)


def build_ir(l1, l2, m1, m2):
    bl = _Builder()
    t = bl.inp
    PI2 = float(np.pi / 2)
    s2 = bl.sin(t('th2')); c2 = bl.sin(t('th2'), PI2)
    s3 = bl.sin(t('th3')); c3 = bl.sin(t('th3'), PI2)
    s4 = bl.sin(t('th4')); c4 = bl.sin(t('th4'), PI2)
    U = bl.mul(c3, c4); V = bl.mul(s3, c4); W = bl.mul(s3, s4); Z = bl.mul(c3, s4)
    c22 = bl.sq(c2, eng='a'); C42 = bl.sq(c4, eng='a')
    c2s4 = bl.mul(c2, s4); s2U = bl.mul(s2, U); P = bl.add(s2U, c2s4)
    c2c4 = bl.mul(c2, c4); s2Z = bl.mul(s2, Z); Q = bl.sub(c2c4, s2Z)
    S2S4 = bl.mul(s2, s4); C2U = bl.mul(c2, U)
    A2 = bl.mul(s2, c2)
    V2 = bl.sq(V, eng='a')
    S3V = bl.mul(s3, V)
    US4 = bl.mul(U, s4, eng='g')
    WS4 = bl.mul(W, s4, eng='g')
    C4S4 = bl.mul(c4, s4, eng='g')
    C4V = bl.mul(c4, V, eng='g')
    C2V = bl.mul(c2, V, eng='g')
    C2C3 = bl.mul(c2, c3, eng='g')
    S2V2 = bl.mul(s2, V2, eng='g')
    S2S3 = bl.mul(s2, s3, eng='g'); S2S3q = bl.sq(S2S3, eng='a')
    Qq = bl.sq(Q, eng='a')
    Rm = bl.sub(C2U, S2S4)
    C2Rm = bl.mul(c2, Rm)
    qq = bl.add(Qq, S2S3q)
    c2t2 = bl.affine(c22, 2.0, -1.0)
    c2t4 = bl.affine(C42, 2.0, -1.0)
    S2P = bl.mul(s2, P)
    K   = bl.sub(bl.smul(S2P, 2*l2), bl.smul(c22, 3*l1))
    E0  = bl.sub(bl.smul(c2, 3*l1), bl.smul(S2S4, 2*l2))
    E2  = bl.add(E0, bl.smul(C2U, 2*l2))
    B   = bl.add(bl.smul(s2, 3*l1), bl.smul(P, 2*l2))
    F1  = bl.affine(U, 2*l2, 3*l1)
    F2  = bl.sub(bl.smul(c3, 3*l1), bl.smul(S3V, 2*l2))
    E02 = bl.add(bl.smul(C2C3, 3*l1), bl.smul(Q, 2*l2))
    aW, bW, cW = 2*l2*l2*m2, 6*l1*l2*m2, 2*l1*l1*(m1+3*m2)
    U2 = bl.sq(U, eng='a')
    w1 = bl.add(bl.smul(U2, aW), bl.smul(U, bW))
    w2 = bl.add(bl.smul(C42, aW), w1)
    W2 = bl.sadd(w2, cW - aW)
    p1 = bl.mul(F1, s4)
    p1c = bl.mul(p1, c2t2)
    p2 = bl.mul(A2, W2)
    GG = bl.add(bl.smul(p1c, l2*m2), p2)
    x1 = bl.mul(c3, F1)
    i1 = bl.add(x1, bl.smul(c4, 2*l2))
    x2 = bl.mul(c3, c2t4)
    i2 = bl.add(bl.smul(x2, 2*l2), bl.smul(c4, 3*l1))
    S4C22 = bl.mul(s4, c22)
    b1 = bl.mul(S4C22, i1)
    b2 = bl.mul(A2, i2)
    C3US4 = bl.mul(c3, US4)
    z1 = bl.add(b1, b2)
    B14 = bl.sub(z1, bl.smul(C3US4, 2*l2))
    VK = bl.mul(V, K, eng='g'); VE2 = bl.mul(V, E2, eng='g'); VE0 = bl.mul(V, E0, eng='g'); VQ = bl.mul(V, Q, eng='g')
    VF1 = bl.mul(V, F1, eng='g'); S4F2 = bl.mul(s4, F2, eng='g'); BU = bl.mul(B, U, eng='g'); BW = bl.mul(B, W, eng='g')
    VB = bl.mul(V, B, eng='g'); S4E02 = bl.mul(s4, E02, eng='g'); C4P = bl.mul(c4, P, eng='g')
    e1 = bl.add(bl.smul(c22, l1*l1*m1/3 + l1*l1*m2), bl.smul(C2Rm, l1*l2*m2))
    d00 = bl.add(e1, bl.smul(qq, l2*l2*m2/3))
    d01 = bl.smul(VB, l2*m2/6)
    d02 = bl.smul(bl.mul(c4, E02), l2*m2/6)
    i3 = bl.add(bl.smul(s2, 2*l2), bl.smul(c2s4, -3*l1))
    d03 = bl.smul(bl.mul(s3, i3), l2*m2/6)
    e3 = bl.add(bl.smul(U, l1*l2*m2), bl.smul(V2, -l2*l2*m2/3))
    d11 = bl.sadd(e3, l1*l1*m1/3 + l1*l1*m2 + l2*l2*m2/3)
    d12 = bl.smul(bl.mul(V, s4, eng='g'), l2*l2*m2/3)
    d13 = bl.add(bl.smul(c4, l1*l2*m2/2), bl.smul(c3, l2*l2*m2/3))
    d22 = bl.smul(C42, l2*l2*m2/3)
    g3 = l2*l2*m2/3
    q = {}
    for i_ in range(1, 5):
        for j_ in range(i_, 5):
            q[(i_, j_)] = bl.mul(t(f'dt{i_}'), t(f'dt{j_}'), eng='g')
    lm, l2m = l2*m2, l2*l2*m2
    hterms = {
     0: [((1,2), GG, -1.0/3), ((1,3), VK, lm/3), ((1,4), B14, -lm/3),
         ((2,2), VE2, lm/6), ((2,3), S2V2, -2*l2m/3), ((2,4), VQ, 2*l2m/3),
         ((3,3), VE0, -lm/6), ((3,4), S4E02, -lm/3), ((4,4), C2V, -l1*lm/2)],
     1: [((1,1), GG, 1.0/6), ((1,3), BU, lm/3), ((1,4), BW, -lm/3),
         ((2,3), VF1, -lm/3), ((2,4), S4F2, -lm/3), ((3,3), US4, l2m/3),
         ((3,4), WS4, -2*l2m/3), ((4,4), s4, -l1*lm/2)],
     2: [((1,1), VK, -lm/6), ((1,2), BU, -lm/3), ((1,4), C4P, -2*l2m/3),
         ((2,2), VF1, lm/6), ((2,4), C4V, 2*l2m/3), ((3,4), C4S4, -2*l2m/3)],
     3: [((1,1), B14, lm/6), ((1,2), BW, lm/3), ((1,3), C4P, 2*l2m/3),
         ((2,2), S4F2, lm/6), ((2,3), C4V, -2*l2m/3), ((3,3), C4S4, l2m/3)],
    }
    rhs = []
    for k in range(4):
        acc = t(f'ta{k+1}')
        for (qp, br, cf) in hterms[k]:
            term = bl.mul(q[qp], br, eng='g')
            acc = bl.add(bl.smul(term, -cf), acc)
        rhs.append(acc)
    a_, b_, c_, d_, e_, f_, g_, h_ = d00, d01, d02, d03, d11, d12, d13, d22
    inv3 = 1.0 / g3
    a00 = bl.sub(a_, bl.smul(bl.sq(d_, eng='a'), inv3))
    a01 = bl.sub(b_, bl.smul(bl.mul(d_, g_), inv3))
    a11 = bl.sub(e_, bl.smul(bl.sq(g_, eng='a'), inv3))
    r2 = bl.recip(h_)
    l02 = bl.mul(c_, r2); l12 = bl.mul(f_, r2)
    b00 = bl.sub(a00, bl.mul(l02, c_))
    b01 = bl.sub(a01, bl.mul(l12, c_))
    b11 = bl.sub(a11, bl.mul(l12, f_))
    r0p = bl.recip(b00)
    l01 = bl.mul(b01, r0p)
    c11 = bl.sub(b11, bl.mul(l01, b01))
    r1p = bl.recip(c11)
    y3, y2 = rhs[3], rhs[2]
    y0 = bl.sub(bl.sub(rhs[0], bl.smul(bl.mul(d_, y3), inv3)), bl.mul(l02, y2))
    y1 = bl.sub(bl.sub(bl.sub(rhs[1], bl.smul(bl.mul(g_, y3), inv3)),
                       bl.mul(l12, y2)), bl.mul(l01, y0))
    z3 = bl.smul(y3, inv3); z2 = bl.mul(y2, r2)
    z0 = bl.mul(y0, r0p);   z1 = bl.mul(y1, r1p)
    x1s = z1
    x0s = bl.sub(z0, bl.mul(l01, x1s))
    x2s = bl.sub(bl.sub(z2, bl.mul(l02, x0s)), bl.mul(l12, x1s))
    x3s = bl.sub(bl.sub(z3, bl.smul(bl.mul(d_, x0s), inv3)),
                 bl.smul(bl.mul(g_, x1s), inv3))
    bl.ops.append(('out_theta',))
    bl.ops.append(('out_tau',))
    for k, xk in enumerate([x0s, x1s, x2s, x3s]):
        bl.ops.append(('out_vel', k, xk.name, DT_STEP * xk.s))
    return bl.ops


def _alloc_registers(ops):
    """Linear-scan register allocation over the IR. Returns (reg_of, nregs)."""
    last_use = {}
    defs = set()
    for i, op in enumerate(ops):
        if op[0] in ENG_OPS:
            defs.add(op[1])
            for a in op[2:]:
                if isinstance(a, str) and a in defs:
                    last_use[a] = i
        elif op[0] == 'out_vel':
            last_use[op[2]] = i
    free = []
    reg_of = {}
    nregs = 0
    live = set()
    for i, op in enumerate(ops):
        if op[0] not in ENG_OPS:
            continue
        for nm in [n for n in live if last_use.get(n, -1) < i]:
            live.discard(nm)
            free.append(reg_of[nm])
        o = op[1]
        if o in last_use:
            if free:
                r = free.pop()
            else:
                r = nregs
                nregs += 1
            reg_of[o] = r
            live.add(o)
    return reg_of, nregs


def _register_const(nc, mybir, value, dtype=None):
    dtype = dtype or mybir.dt.float32
    t = nc.alloc_sbuf_tensor(f"const-{dtype.name}-{value}", [128, 1], dtype)
    nc.gpsimd.memset(t.ap(), value)
    nc.const_aps.aps[(dtype, value)] = t.ap()


def build_kernel(l1, l2, m1, m2):
    import sys
    if '/opt/trn_rl_repo' not in sys.path:
        sys.path.insert(0, '/opt/trn_rl_repo')
    from concourse import bacc, mybir, tile

    ops = build_ir(l1, l2, m1, m2)
    reg_of, nregs = _alloc_registers(ops)

    nc = bacc.Bacc(None)
    F32 = mybir.dt.float32
    A = mybir.AluOpType
    AF = mybir.ActivationFunctionType

    _register_const(nc, mybir, float(np.pi / 2))
    nc.all_engine_barrier()

    theta_d = nc.declare_dram_parameter("theta", [PADDED, 4], F32, isOutput=False)
    vel_d = nc.declare_dram_parameter("vel", [PADDED, 4], F32, isOutput=False)
    tau_d = nc.declare_dram_parameter("tau", [PADDED, 4], F32, isOutput=False)
    out_d = nc.declare_dram_parameter("out", [PADDED, 12], F32, isOutput=True)

    theta_r = theta_d[:].rearrange("(p r) c -> p r c", p=128)
    vel_r = vel_d[:].rearrange("(p r) c -> p r c", p=128)
    tau_r = tau_d[:].rearrange("(p r) c -> p r c", p=128)
    out_r = out_d[:].rearrange("(p r) c -> p r c", p=128)

    FMAX = max(CHUNKS)

    with tile.TileContext(nc) as tc:
        with tc.tile_pool(name="io", bufs=2) as iop, \
             tc.tile_pool(name="work", bufs=1) as wp:
            off = 0
            for F in CHUNKS:
                th_t = iop.tile([128, FMAX * 4], F32, tag="th")
                ve_t = iop.tile([128, FMAX * 4], F32, tag="ve")
                ta_t = iop.tile([128, FMAX * 4], F32, tag="ta")
                ou_t = iop.tile([128, FMAX * 12], F32, tag="ou")
                th_v = th_t[:].rearrange("p (r c) -> p r c", c=4)[:, :F, :]
                ve_v = ve_t[:].rearrange("p (r c) -> p r c", c=4)[:, :F, :]
                ta_v = ta_t[:].rearrange("p (r c) -> p r c", c=4)[:, :F, :]
                ou_v = ou_t[:].rearrange("p (r c) -> p r c", c=12)[:, :F, :]
                nc.sync.dma_start(out=th_v, in_=theta_r[:, off:off + F, :])
                nc.sync.dma_start(out=ve_v, in_=vel_r[:, off:off + F, :])
                nc.sync.dma_start(out=ta_v, in_=tau_r[:, off:off + F, :])

                regs = {}
                def rtile(name):
                    r = reg_of[name]
                    if r not in regs:
                        regs[r] = wp.tile([128, FMAX], F32, tag=f"r{r}",
                                          name=f"r{r}")
                    return regs[r][:, :F]

                def get(name):
                    if name.startswith('th'):
                        return th_v[:, :, int(name[2]) - 1]
                    if name.startswith('dt'):
                        return ve_v[:, :, int(name[2]) - 1]
                    if name.startswith('ta'):
                        return ta_v[:, :, int(name[2]) - 1]
                    return rtile(name)

                for op in ops:
                    tag = op[0]
                    if tag == 'sin':
                        _, o, a, bias, _e = op
                        nc.scalar.activation(rtile(o), get(a), AF.Sin,
                                             bias=float(bias))
                    elif tag == 'sq':
                        _, o, a, _e = op
                        nc.scalar.activation(rtile(o), get(a), AF.Square)
                    elif tag in ('mul', 'add', 'sub'):
                        _, o, a, b, e = op
                        alu = {'mul': A.mult, 'add': A.add, 'sub': A.subtract}[tag]
                        eng = nc.gpsimd if e == 'g' else nc.vector
                        eng.tensor_tensor(out=rtile(o), in0=get(a),
                                          in1=get(b), op=alu)
                    elif tag == 'stt':
                        _, o, a, s, op0, b, op1, _e = op
                        nc.vector.scalar_tensor_tensor(
                            out=rtile(o), in0=get(a), scalar=float(s),
                            in1=get(b), op0=getattr(A, op0), op1=getattr(A, op1))
                    elif tag == 'ts':
                        _, o, a, s1, op0, s2, op1, e = op
                        if e == 'a' and op0 == 'mult':
                            nc.scalar.activation(
                                rtile(o), get(a), AF.Copy,
                                bias=0.0 if op1 is None else float(s2),
                                scale=float(s1))
                        elif op1 is None:
                            nc.vector.tensor_scalar(
                                out=rtile(o), in0=get(a), scalar1=float(s1),
                                scalar2=None, op0=getattr(A, op0))
                        else:
                            nc.vector.tensor_scalar(
                                out=rtile(o), in0=get(a), scalar1=float(s1),
                                scalar2=float(s2), op0=getattr(A, op0),
                                op1=getattr(A, op1))
                    elif tag == 'recip':
                        _, o, a, _e = op
                        nc.vector.reciprocal_approx_fast(out=rtile(o), in_=get(a))
                    elif tag == 'out_theta':
                        nc.vector.scalar_tensor_tensor(
                            out=ou_v[:, :, 0:4], in0=ve_v, scalar=DT_STEP,
                            in1=th_v, op0=A.mult, op1=A.add)
                    elif tag == 'out_tau':
                        nc.vector.tensor_scalar(
                            out=ou_v[:, :, 8:12], in0=ta_v,
                            scalar1=float(1.0 - LAM * DT_STEP), scalar2=None,
                            op0=A.mult)
                    elif tag == 'out_vel':
                        _, k, node, s = op
                        nc.vector.scalar_tensor_tensor(
                            out=ou_v[:, :, 4 + k], in0=get(node),
                            scalar=float(s), in1=ve_v[:, :, k],
                            op0=A.mult, op1=A.add)
                    else:
                        raise ValueError(tag)

                nc.sync.dma_start(out=out_r[:, off:off + F, :], in_=ou_v)
                off += F

    nc.finalize()
    return nc


_cache = {}


def _get_nc(l1, l2, m1, m2):
    key = (round(l1, 9), round(l2, 9), round(m1, 9), round(m2, 9))
    if key not in _cache:
        _cache[key] = build_kernel(l1, l2, m1, m2)
    return _cache[key]


def _shard_inputs(theta, vel, tau):
    in_maps = []
    for c in range(NCORES):
        m = {}
        for name, arr in (("theta", theta), ("vel", vel), ("tau", tau)):
            a = np.asarray(arr, dtype=np.float32)[c * ROWS_PER_CORE:(c + 1) * ROWS_PER_CORE]
            p = np.zeros((PADDED, 4), np.float32)
            p[:ROWS_PER_CORE] = a
            m[name] = p
        in_maps.append(m)
    return in_maps


def _run(nc, in_maps, trace=False, **kw):
    import sys
    if '/opt/trn_rl_repo' not in sys.path:
        sys.path.insert(0, '/opt/trn_rl_repo')
    from concourse.bass_utils import run_bass_kernel_spmd
    return run_bass_kernel_spmd(nc, in_maps, core_ids=list(range(NCORES)),
                                trace=trace, **kw)


def kernel(theta, vel, tau, L1, L2, M1, M2):
    l1 = float(np.asarray(L1).ravel()[0])
    l2 = float(np.asarray(L2).ravel()[0])
    m1 = float(np.asarray(M1).ravel()[0])
    m2 = float(np.asarray(M2).ravel()[0])
    nc = _get_nc(l1, l2, m1, m2)
    in_maps = _shard_inputs(theta, vel, tau)
    res = _run(nc, in_maps)
    out = np.concatenate(
        [res.results[c]["out"][:ROWS_PER_CORE] for c in range(NCORES)], axis=0)
    return out.astype(np.float32)
